# revision 1
# baseline (speedup 1.0000x reference)
"""ArgmaxIOU kernel v6 for 8 Trainium2 NeuronCores.

Data-parallel over batch: core i processes sample i (B=8, C=21, H=W=512).
Raw Bass (explicit engines + semaphores).

v6 = v5 + tail surgery (v5 lost ~16.7us after the last DMA byte):
 - prediction tile 3's last class-quarter is loaded in column halves,
   so after the final byte only half the tile's Q4-fold + eq remains
   (~4.3us of DVE instead of ~8.7us)
 - the PSUM->SBUF extract moved from ACT to DVE: drops the 1.3us
   ACT_TABLE_LOAD that sat on the critical path before the final store
 - v5 recap: 4x512-col tiles (21.5K descriptors, cured the SDMA
   engine-15 descriptor-ring straggle), class-quarter-split tiles 0/3,
   flat contiguous [C,512] slots, [TB,C,G] eq masks, 128-col eq pieces
   on the last tile, HWDGE final store.

Host: gather 8 packed [84,84] matrices, fold G, compute mean IoU.
"""

import sys

import numpy as np

for p in ("/opt/trn_rl_repo",):
    if p not in sys.path:
        sys.path.insert(0, p)

from contextlib import ExitStack

from concourse import bass, mybir
from concourse.bass_utils import run_bass_kernel_spmd

B = 8
C = 21
HW = 512 * 512
P = 128
Q = HW // P              # 2048 pixel columns per partition
G = 4
M = G * C                # 84
EQS = 256                # eq slot columns
TBS = EQS // G           # 64 matmul blocks per eq slot
NEQ = 3
NSLOT = 3
SLOTE = C * 512

CH = [512, 512, 512, 512]
OFF = [0, 512, 1024, 1536]
SPLIT = {0, 3}                              # class-quarter-split tiles
QTR = [(0, 5), (5, 10), (10, 15), (15, 21)]

F32 = mybir.dt.float32
BF16 = mybir.dt.bfloat16


def build():
    nc = bass.Bass()
    pred = nc.declare_dram_parameter("prediction", [C, HW], F32, isOutput=False)
    targ = nc.declare_dram_parameter("target", [C, HW], F32, isOutput=False)
    out = nc.declare_dram_parameter("out", [M, M], F32, isOutput=True)

    predv = pred[:].rearrange("c (p q) -> p c q", p=P)
    targv = targ[:].rearrange("c (p q) -> p c q", p=P)

    mx = mybir.AluOpType.max
    eqop = mybir.AluOpType.is_equal
    cp = mybir.ActivationFunctionType.Copy

    NT = len(CH)

    # one sem per LOAD (DMA completions are not ordered across loads):
    # split tiles get 4 quarter-sems, full tiles one; pred tile 3's Q4
    # is column-halved -> 5 sems
    skeys = []
    for k in range(NT):
        for tid in (0, 1):
            if k == NT - 1 and tid == 1:
                skeys += [(k, tid, qi) for qi in range(5)]
            elif k in SPLIT:
                skeys += [(k, tid, qi) for qi in range(4)]
            else:
                skeys += [(k, tid, 0)]
    sidx = {key: i for i, key in enumerate(skeys)}

    # eq pieces: (col_lo, col_len); last tile gets 128-col pieces
    def eq_pieces(k):
        if k == NT - 1:
            return [(q * 128, 128) for q in range(4)]
        return [(0, EQS), (EQS, EQS)]

    eq_pos_last = {}
    eq_pos_piece = {}
    sub_of = {}
    pos = 0
    subctr = {0: 0, 1: 0}
    for k in range(NT):
        pieces = eq_pieces(k)
        for tid in (0, 1):
            eq_pos_piece[(k, tid)] = []
            sub_of[(k, tid)] = []
            base = subctr[tid]
            for (lo, ln) in pieces:
                pos += 1
                eq_pos_piece[(k, tid)].append(pos)
                sub_of[(k, tid)].append(base + lo // EQS)
            subctr[tid] = base + CH[k] // EQS
            eq_pos_last[(k, tid)] = pos
    NSUB = subctr[0]                        # 8
    n_eq_ops = pos

    ring_gate = {}
    for k in range(NSLOT, NT):
        for tid in (0, 1):
            ring_gate[(k, tid)] = eq_pos_last[(k - NSLOT, tid)]

    pe_sched = {j: [] for j in range(NSUB)}
    for k in range(NT):
        for i, (lo, ln) in enumerate(eq_pieces(k)):
            j = sub_of[(k, 1)][i]
            pe_sched[j].append(((lo % EQS) // G, ln // G,
                                eq_pos_piece[(k, 1)][i]))

    with ExitStack() as ctx:
        e_ = ctx.enter_context
        bt = e_(nc.sbuf_tensor("bt", [P, NSLOT * SLOTE], BF16))
        bp = e_(nc.sbuf_tensor("bp", [P, NSLOT * SLOTE], BF16))
        eqt = e_(nc.sbuf_tensor("eqt", [P, NEQ, TBS, C, G], BF16))
        eqp = e_(nc.sbuf_tensor("eqp", [P, NEQ, TBS, C, G], BF16))
        st = e_(nc.sbuf_tensor("st", [P, 10, 512], BF16))
        mo = e_(nc.sbuf_tensor("mo", [P, 512], BF16))
        osb = e_(nc.sbuf_tensor("osb", [M, M], F32))
        conf = e_(nc.psum_tensor("conf", [M, M], F32))
        dms = [e_(nc.semaphore(f"dm{i}")) for i in range(len(skeys))]
        dve = e_(nc.semaphore("dve"))
        mm = e_(nc.semaphore("mm"))
        fin = e_(nc.semaphore("fin"))
        block = e_(nc.Block())

        srcs = {0: targv, 1: predv}
        pools = {0: bt, 1: bp}

        def tile_view(tid, k):
            s = k % NSLOT
            return pools[tid][:, s * SLOTE:(s + 1) * SLOTE] \
                .rearrange("p (c t) -> p c t", c=C)

        @block.gpsimd
        def _(g):
            for k in range(NT):
                for tid in (0, 1):
                    gate = ring_gate.get((k, tid))
                    if gate is not None:
                        g.wait_ge(dve, gate)
                    tv = tile_view(tid, k)
                    if k == NT - 1 and tid == 1:
                        # Q1-3 full-width; Q4 in column halves (tail)
                        parts = [(0, 5, 0, 512), (5, 10, 0, 512),
                                 (10, 15, 0, 512), (15, 21, 0, 256),
                                 (15, 21, 256, 512)]
                        for qi, (clo, chi, tlo, thi) in enumerate(parts):
                            g.dma_start(
                                out=tv[:, clo:chi, tlo:thi],
                                in_=srcs[tid][:, clo:chi,
                                              OFF[k] + tlo:OFF[k] + thi],
                            ).then_inc(dms[sidx[(k, tid, qi)]], 16)
                        continue
                    halves = QTR if k in SPLIT else [(0, C)]
                    for qi, (clo, chi) in enumerate(halves):
                        g.dma_start(
                            out=tv[:, clo:chi, :],
                            in_=srcs[tid][:, clo:chi, OFF[k]:OFF[k] + CH[k]],
                        ).then_inc(dms[sidx[(k, tid, qi)]], 16)

        @block.vector
        def _(v):
            def tree_quarters(data, k, tid):
                # 4-stage tree as class quarters arrive (per-quarter sems):
                # partials in st rows 0, 2, 4, 6; fold -> mo
                v.wait_ge(dms[sidx[(k, tid, 0)]], 16)
                v.tensor_tensor(st[:, 0:2], data[:, 0:2, :], data[:, 2:4, :], mx)
                v.tensor_tensor(st[:, 0:1], st[:, 0:1], st[:, 1:2], mx)
                v.tensor_tensor(st[:, 0:1], st[:, 0:1], data[:, 4:5, :], mx)
                v.wait_ge(dms[sidx[(k, tid, 1)]], 16)
                v.tensor_tensor(st[:, 2:4], data[:, 5:7, :], data[:, 7:9, :], mx)
                v.tensor_tensor(st[:, 2:3], st[:, 2:3], st[:, 3:4], mx)
                v.tensor_tensor(st[:, 2:3], st[:, 2:3], data[:, 9:10, :], mx)
                v.wait_ge(dms[sidx[(k, tid, 2)]], 16)
                v.tensor_tensor(st[:, 4:6], data[:, 10:12, :], data[:, 12:14, :], mx)
                v.tensor_tensor(st[:, 4:5], st[:, 4:5], st[:, 5:6], mx)
                v.tensor_tensor(st[:, 4:5], st[:, 4:5], data[:, 14:15, :], mx)
                v.wait_ge(dms[sidx[(k, tid, 3)]], 16)
                v.tensor_tensor(st[:, 6:9], data[:, 15:18, :], data[:, 18:21, :], mx)
                v.tensor_tensor(st[:, 6:7], st[:, 6:7], st[:, 7:8], mx)
                v.tensor_tensor(st[:, 6:7], st[:, 6:7], st[:, 8:9], mx)
                v.tensor_tensor(st[:, 0:1], st[:, 0:1], st[:, 2:3], mx)
                v.tensor_tensor(st[:, 4:5], st[:, 4:5], st[:, 6:7], mx)
                v.tensor_tensor(mo[:].unsqueeze(1), st[:, 0:1], st[:, 4:5], mx)

            def tree21(data, sem):
                v.wait_ge(sem, 16)
                v.tensor_tensor(st[:, 0:10], data[:, 0:10, :],
                                data[:, 10:20, :], mx)
                v.tensor_tensor(st[:, 0:5], st[:, 0:5], st[:, 5:10], mx)
                v.tensor_tensor(st[:, 0:2], st[:, 0:2], st[:, 2:4], mx)
                v.tensor_tensor(st[:, 0:1], st[:, 0:1], st[:, 1:2], mx)
                v.tensor_tensor(st[:, 0:1], st[:, 0:1], st[:, 4:5], mx)
                v.tensor_tensor(mo[:].unsqueeze(1), st[:, 0:1],
                                data[:, 20:21, :], mx)

            def eq_piece(eqb, data, j, lo, ln):
                e = j % NEQ
                b0 = (lo % EQS) // G
                nb = ln // G
                v.tensor_tensor(
                    eqb[:, e, b0:b0 + nb]
                        .rearrange("p tb c g -> p c tb g"),
                    data[:, :, lo:lo + ln]
                        .rearrange("p c (tb g) -> p c tb g", g=G),
                    mo[:, lo:lo + ln]
                        .rearrange("p (tb g) -> p tb g", g=G)
                        .unsqueeze(1).broadcast_to((P, C, nb, G)),
                    eqop).then_inc(dve, 1)

            def p3_tail(data):
                # pred tile 3: Q1-3 partials full-width; Q4 + fold + eq
                # per column half so only half remains after the last byte
                v.wait_ge(dms[sidx[(3, 1, 0)]], 16)
                v.tensor_tensor(st[:, 0:2], data[:, 0:2, :], data[:, 2:4, :], mx)
                v.tensor_tensor(st[:, 0:1], st[:, 0:1], st[:, 1:2], mx)
                v.tensor_tensor(st[:, 0:1], st[:, 0:1], data[:, 4:5, :], mx)
                v.wait_ge(dms[sidx[(3, 1, 1)]], 16)
                v.tensor_tensor(st[:, 2:4], data[:, 5:7, :], data[:, 7:9, :], mx)
                v.tensor_tensor(st[:, 2:3], st[:, 2:3], st[:, 3:4], mx)
                v.tensor_tensor(st[:, 2:3], st[:, 2:3], data[:, 9:10, :], mx)
                v.wait_ge(dms[sidx[(3, 1, 2)]], 16)
                v.tensor_tensor(st[:, 4:6], data[:, 10:12, :], data[:, 12:14, :], mx)
                v.tensor_tensor(st[:, 4:5], st[:, 4:5], st[:, 5:6], mx)
                v.tensor_tensor(st[:, 4:5], st[:, 4:5], data[:, 14:15, :], mx)
                for h, (tlo, thi) in enumerate(((0, 256), (256, 512))):
                    v.wait_ge(dms[sidx[(3, 1, 3 + h)]], 16)
                    v.tensor_tensor(st[:, 6:9, tlo:thi],
                                    data[:, 15:18, tlo:thi],
                                    data[:, 18:21, tlo:thi], mx)
                    v.tensor_tensor(st[:, 6:7, tlo:thi],
                                    st[:, 6:7, tlo:thi],
                                    st[:, 7:8, tlo:thi], mx)
                    v.tensor_tensor(st[:, 6:7, tlo:thi],
                                    st[:, 6:7, tlo:thi],
                                    st[:, 8:9, tlo:thi], mx)
                    # non-destructive fold (row 9) -> mo[tlo:thi]
                    v.tensor_tensor(st[:, 9:10, tlo:thi],
                                    st[:, 0:1, tlo:thi],
                                    st[:, 2:3, tlo:thi], mx)
                    v.tensor_tensor(st[:, 9:10, tlo:thi],
                                    st[:, 9:10, tlo:thi],
                                    st[:, 4:5, tlo:thi], mx)
                    v.tensor_tensor(mo[:, tlo:thi].unsqueeze(1),
                                    st[:, 9:10, tlo:thi],
                                    st[:, 6:7, tlo:thi], mx)
                    j = 6 + h
                    v.wait_ge(mm, j - NEQ + 1)
                    eq_piece(eqp, data, j, tlo, 128)
                    eq_piece(eqp, data, j, tlo + 128, 128)

            for k in range(NT):
                for tid in (0, 1):
                    data = tile_view(tid, k)
                    eqb = eqt if tid == 0 else eqp
                    if k == NT - 1 and tid == 1:
                        p3_tail(data)
                        continue
                    if k in SPLIT:
                        tree_quarters(data, k, tid)
                    else:
                        tree21(data, dms[sidx[(k, tid, 0)]])
                    for i, (lo, ln) in enumerate(eq_pieces(k)):
                        j = sub_of[(k, tid)][i]
                        new_sub = i == 0 or sub_of[(k, tid)][i - 1] != j
                        if j >= NEQ and new_sub:
                            v.wait_ge(mm, j - NEQ + 1)
                        eq_piece(eqb, data, j, lo, ln)

            # PSUM -> SBUF extract on DVE (ACT's table load would cost
            # ~1.3us on the critical path)
            v.wait_ge(mm, NSUB)
            v.tensor_scalar_add(osb[:], conf[:], 0.0).then_inc(dve, 1)

        @block.tensor
        def _(te):
            first = True
            for j in range(NSUB):
                e = j % NEQ
                for (blo, bn, wval) in pe_sched[j]:
                    te.wait_ge(dve, wval)
                    for tb in range(blo, blo + bn):
                        inst = te.matmul(
                            conf[:],
                            eqt[:, e, tb].rearrange("p c g -> p (c g)"),
                            eqp[:, e, tb].rearrange("p c g -> p (c g)"),
                            start=first,
                            stop=(j == NSUB - 1
                                  and (blo, bn, wval) == pe_sched[j][-1]
                                  and tb == blo + bn - 1))
                        first = False
                inst.then_inc(mm, 1)

        @block.sync
        def _(sy):
            sy.wait_ge(dve, n_eq_ops + 1)
            sy.dma_start(out=out[:], in_=osb[:]).then_inc(fin, 16)
            sy.wait_ge(fin, 16)

    return nc


def _score_from_packed(packed):
    """packed: [84, 84] f32 -> per-sample mean IoU (float64)."""
    x = packed.astype(np.float64).reshape(C, G, C, G)
    conf = np.einsum("igjg->ij", x)
    TP = np.diag(conf).copy()
    FN = conf.sum(axis=1) - TP
    FP = conf.sum(axis=0) - TP
    valid = TP > 0
    denom = TP + FN + FP
    iou = np.where(valid, TP / np.where(valid, denom, 1.0), 0.0)
    n_valid = max(float(valid.sum()), 1.0)
    return iou.sum() / n_valid


_NC_CACHE = {}


def _get_nc():
    if "nc" not in _NC_CACHE:
        _NC_CACHE["nc"] = build()
    return _NC_CACHE["nc"]


def run(prediction, target, trace=False):
    in_maps = []
    for i in range(B):
        in_maps.append({
            "prediction": np.ascontiguousarray(
                np.asarray(prediction[i], dtype=np.float32).reshape(C, HW)),
            "target": np.ascontiguousarray(
                np.asarray(target[i], dtype=np.float32).reshape(C, HW)),
        })
    res = run_bass_kernel_spmd(_get_nc(), in_maps, core_ids=list(range(B)),
                               trace=trace)
    scores = [_score_from_packed(res.results[i]["out"]) for i in range(B)]
    return np.float32(np.mean(scores)), res


def kernel(prediction, target):
    score, _ = run(prediction, target, trace=False)
    return score



# revision 2
# speedup vs baseline: 1.0094x; 1.0094x over previous
"""ArgmaxIOU kernel v9 for 8 Trainium2 NeuronCores.

Data-parallel over batch: core i processes sample i (B=8, C=21, H=W=512).
Raw Bass (explicit engines + semaphores).

v8 = DVE op-count diet (v7 trace showed DMA at 387 GB/s and DVE as the
new bottleneck at 127.7us busy, bloated by ~110 overhead-dominated
narrow-tile ops):
 - tree21 (6 wide tensor_tensor ops, one sem wait) for every tile
   except the last; quarter-tree only on the final 128-col tile where
   the post-last-byte latency needs it.
 - 512-col eq ops that fill both eq ring slots (subs 2k, 2k+1) in one
   instruction.
 - tiles (512, 512, 512, 384, 128); tile 0 emitted as a small 2-class
   load first so the SDMA doorbell rings early.

Host: gather 8 packed [84,84] matrices, fold G, compute mean IoU.
"""

import sys

import numpy as np

for p in ("/opt/trn_rl_repo",):
    if p not in sys.path:
        sys.path.insert(0, p)

from contextlib import ExitStack

from concourse import bass, mybir
from concourse.bass_utils import run_bass_kernel_spmd

B = 8
C = 21
HW = 512 * 512
P = 128
Q = HW // P              # 2048 pixel columns per partition
G = 4
M = G * C                # 84
EQS = 256                # eq sub-slot columns
TBS = EQS // G           # 64 matmul blocks per eq sub-slot
NEQ = 4                  # eq slot ring depth (even: 512-col eq pair-writes)
NSLOT = 2                # data tile ring depth
SLOTW = 512              # slot width (max tile width)
SLOTE = C * SLOTW

W = [512, 512, 512, 384, 128]
OFF = [0, 512, 1024, 1536, 1920]
NT = len(W)
LAST = NT - 1
QTR = [(0, 5), (5, 10), (10, 15), (15, 21)]

F32 = mybir.dt.float32
BF16 = mybir.dt.bfloat16


def build():
    nc = bass.Bass()
    pred = nc.declare_dram_parameter("prediction", [C, HW], F32, isOutput=False)
    targ = nc.declare_dram_parameter("target", [C, HW], F32, isOutput=False)
    out = nc.declare_dram_parameter("out", [M, M], F32, isOutput=True)

    predv = pred[:].rearrange("c (p q) -> p c q", p=P)
    targv = targ[:].rearrange("c (p q) -> p c q", p=P)

    mx = mybir.AluOpType.max
    eqop = mybir.AluOpType.is_equal

    # ---- load bookkeeping: one sem per load ----
    # tile 0 target: 2 loads (classes 0-2 doorbell + 2-21); last tile:
    # class quarters; everything else: one full-tile load.
    def loads_of(k, tid):
        if k == 0 and tid == 0:
            return 2
        if k == LAST:
            return 2
        return 1

    skeys = []
    for k in range(NT):
        for tid in (0, 1):
            skeys += [(k, tid, qi) for qi in range(loads_of(k, tid))]
    sidx = {key: i for i, key in enumerate(skeys)}

    # ---- eq piece bookkeeping (global columns) ----
    pieces_of = {
        0: [(0, 512)],
        1: [(512, 512)],
        2: [(1024, 512)],
        3: [(1536, 256), (1792, 128)],
        4: [(1920, 96), (2016, 32)],
    }

    eqpos = {}
    pos = 0
    for k in range(NT):
        for tid in (0, 1):
            for i in range(len(pieces_of[k])):
                pos += 1
                eqpos[(k, tid, i)] = pos
    n_eq = pos

    eq_done = {k: eqpos[(k, 1, len(pieces_of[k]) - 1)] for k in range(NT)}
    ring_gate = {k: eq_done[k - NSLOT] for k in range(NSLOT, NT)}

    # PE schedule: per sub j, list of (b0, nb, dve_gate)
    NSUB = Q // EQS                      # 8
    pe_sched = {j: [] for j in range(NSUB)}
    for k in range(NT):
        for i, (gcol, glen) in enumerate(pieces_of[k]):
            gate = eqpos[(k, 1, i)]
            j0 = gcol // EQS
            j1 = (gcol + glen - 1) // EQS
            for j in range(j0, j1 + 1):
                lo = max(gcol, j * EQS)
                hi = min(gcol + glen, (j + 1) * EQS)
                pe_sched[j].append(((lo % EQS) // G, (hi - lo) // G, gate))

    with ExitStack() as ctx:
        e_ = ctx.enter_context
        bt = e_(nc.sbuf_tensor("bt", [P, NSLOT * SLOTE], BF16))
        bp = e_(nc.sbuf_tensor("bp", [P, NSLOT * SLOTE], BF16))
        eqt = e_(nc.sbuf_tensor("eqt", [P, NEQ, TBS, C, G], BF16))
        eqp = e_(nc.sbuf_tensor("eqp", [P, NEQ, TBS, C, G], BF16))
        st = e_(nc.sbuf_tensor("st", [P, 10, SLOTW], BF16))
        mo = e_(nc.sbuf_tensor("mo", [P, SLOTW], BF16))
        osb = e_(nc.sbuf_tensor("osb", [M, M], F32))
        conf = e_(nc.psum_tensor("conf", [M, M], F32))
        dms = [e_(nc.semaphore(f"dm{i}")) for i in range(len(skeys))]
        dve = e_(nc.semaphore("dve"))
        mm = e_(nc.semaphore("mm"))
        fin = e_(nc.semaphore("fin"))
        block = e_(nc.Block())

        srcs = {0: targv, 1: predv}
        pools = {0: bt, 1: bp}

        def tile_view(tid, k):
            s = k % NSLOT
            return pools[tid][:, s * SLOTE:(s + 1) * SLOTE] \
                .rearrange("p (c t) -> p c t", c=C)

        @block.gpsimd
        def _(g):
            def emit(k, tid):
                tv = tile_view(tid, k)
                w = W[k]
                if k == 0 and tid == 0:
                    for qi, (clo, chi) in enumerate([(0, 2), (2, 21)]):
                        g.dma_start(
                            out=tv[:, clo:chi, 0:w],
                            in_=srcs[tid][:, clo:chi, OFF[k]:OFF[k] + w],
                        ).then_inc(dms[sidx[(k, tid, qi)]], 16)
                elif k == LAST:
                    for qi, (clo, chi) in enumerate([(0, 15), (15, 21)]):
                        g.dma_start(
                            out=tv[:, clo:chi, 0:w],
                            in_=srcs[tid][:, clo:chi, OFF[k]:OFF[k] + w],
                        ).then_inc(dms[sidx[(k, tid, qi)]], 16)
                else:
                    g.dma_start(
                        out=tv[:, :, 0:w],
                        in_=srcs[tid][:, :, OFF[k]:OFF[k] + w],
                    ).then_inc(dms[sidx[(k, tid, 0)]], 16)

            for k in range(NSLOT):
                for tid in (0, 1):
                    emit(k, tid)
            for k in range(NSLOT, NT):
                g.wait_ge(dve, ring_gate[k])
                emit(k, 0)
                emit(k, 1)

        @block.vector
        def _(v):
            def tree21(k, tid):
                # full tile landed: 6 wide ops -> mo[0:w]
                data = tile_view(tid, k)
                w = W[k]
                for qi in range(loads_of(k, tid)):
                    v.wait_ge(dms[sidx[(k, tid, qi)]], 16)
                v.tensor_tensor(st[:, 0:10, 0:w], data[:, 0:10, 0:w],
                                data[:, 10:20, 0:w], mx)
                v.tensor_tensor(st[:, 0:5, 0:w], st[:, 0:5, 0:w], st[:, 5:10, 0:w], mx)
                v.tensor_tensor(st[:, 0:2, 0:w], st[:, 0:2, 0:w], st[:, 2:4, 0:w], mx)
                v.tensor_tensor(st[:, 0:1, 0:w], st[:, 0:1, 0:w], st[:, 1:2, 0:w], mx)
                v.tensor_tensor(st[:, 0:1, 0:w], st[:, 0:1, 0:w], st[:, 4:5, 0:w], mx)
                v.tensor_tensor(mo[:, 0:w].unsqueeze(1), st[:, 0:1, 0:w],
                                data[:, 20:21, 0:w], mx)

            def tree_tail(k, tid):
                # last tile: classes 0-14 folded while the final 6-class
                # load is in flight; only 4 small ops after the last byte
                data = tile_view(tid, k)
                w = W[k]
                v.wait_ge(dms[sidx[(k, tid, 0)]], 16)
                v.tensor_tensor(st[:, 0:7, 0:w], data[:, 0:7, 0:w], data[:, 7:14, 0:w], mx)
                v.tensor_tensor(st[:, 0:3, 0:w], st[:, 0:3, 0:w], st[:, 3:6, 0:w], mx)
                v.tensor_tensor(st[:, 0:1, 0:w], st[:, 0:1, 0:w], st[:, 1:2, 0:w], mx)
                v.tensor_tensor(st[:, 0:1, 0:w], st[:, 0:1, 0:w], st[:, 2:3, 0:w], mx)
                v.tensor_tensor(st[:, 0:1, 0:w], st[:, 0:1, 0:w], st[:, 6:7, 0:w], mx)
                v.tensor_tensor(st[:, 0:1, 0:w], st[:, 0:1, 0:w], data[:, 14:15, 0:w], mx)
                v.wait_ge(dms[sidx[(k, tid, 1)]], 16)
                v.tensor_tensor(st[:, 1:4, 0:w], data[:, 15:18, 0:w], data[:, 18:21, 0:w], mx)
                v.tensor_tensor(st[:, 1:2, 0:w], st[:, 1:2, 0:w], st[:, 2:3, 0:w], mx)
                v.tensor_tensor(st[:, 1:2, 0:w], st[:, 1:2, 0:w], st[:, 3:4, 0:w], mx)
                v.tensor_tensor(mo[:, 0:w].unsqueeze(1), st[:, 0:1, 0:w],
                                st[:, 1:2, 0:w], mx)

            seen_sub = set()

            def eq_piece(k, tid, i):
                gcol, glen = pieces_of[k][i]
                j0 = gcol // EQS
                j1 = (gcol + glen - 1) // EQS
                need = max((j - (NEQ - 1) for j in range(j0, j1 + 1)
                            if j >= NEQ and j not in seen_sub), default=None)
                for j in range(j0, j1 + 1):
                    seen_sub.add(j)
                if need is not None:
                    v.wait_ge(mm, need)
                data = tile_view(tid, k)
                eqb = eqt if tid == 0 else eqp
                lo = gcol - OFF[k]
                if glen == 512:
                    # spans two adjacent eq slots (subs j0 even, j0+1)
                    ep = j0 % NEQ
                    outv = eqb[:, ep:ep + 2].rearrange("p e tb c g -> p c (e tb) g")
                else:
                    e = j0 % NEQ
                    b0 = (gcol % EQS) // G
                    nb = glen // G
                    outv = eqb[:, e, b0:b0 + nb].rearrange("p tb c g -> p c tb g")
                v.tensor_tensor(
                    outv,
                    data[:, :, lo:lo + glen]
                        .rearrange("p c (tb g) -> p c tb g", g=G),
                    mo[:, lo:lo + glen]
                        .rearrange("p (tb g) -> p tb g", g=G)
                        .unsqueeze(1).broadcast_to((P, C, glen // G, G)),
                    eqop).then_inc(dve, 1)

            for k in range(NT):
                for tid in (0, 1):
                    if k == LAST:
                        tree_tail(k, tid)
                    else:
                        tree21(k, tid)
                    for i in range(len(pieces_of[k])):
                        eq_piece(k, tid, i)

            v.wait_ge(mm, NSUB)
            v.tensor_scalar_add(osb[:], conf[:], 0.0).then_inc(dve, 1)

        @block.tensor
        def _(te):
            first = True
            for j in range(NSUB):
                e = j % NEQ
                for (b0, nb, gate) in pe_sched[j]:
                    te.wait_ge(dve, gate)
                    for tb in range(b0, b0 + nb):
                        inst = te.matmul(
                            conf[:],
                            eqt[:, e, tb].rearrange("p c g -> p (c g)"),
                            eqp[:, e, tb].rearrange("p c g -> p (c g)"),
                            start=first,
                            stop=(j == NSUB - 1
                                  and (b0, nb, gate) == pe_sched[j][-1]
                                  and tb == b0 + nb - 1))
                        first = False
                inst.then_inc(mm, 1)

        @block.sync
        def _(sy):
            sy.wait_ge(dve, n_eq + 1)
            sy.dma_start(out=out[:], in_=osb[:]).then_inc(fin, 16)
            sy.wait_ge(fin, 16)

    return nc


def _score_from_packed(packed):
    """packed: [84, 84] f32 -> per-sample mean IoU (float64)."""
    x = packed.astype(np.float64).reshape(C, G, C, G)
    conf = np.einsum("igjg->ij", x)
    TP = np.diag(conf).copy()
    FN = conf.sum(axis=1) - TP
    FP = conf.sum(axis=0) - TP
    valid = TP > 0
    denom = TP + FN + FP
    iou = np.where(valid, TP / np.where(valid, denom, 1.0), 0.0)
    n_valid = max(float(valid.sum()), 1.0)
    return iou.sum() / n_valid


_NC_CACHE = {}


def _get_nc():
    if "nc" not in _NC_CACHE:
        _NC_CACHE["nc"] = build()
    return _NC_CACHE["nc"]


def run(prediction, target, trace=False):
    in_maps = []
    for i in range(B):
        in_maps.append({
            "prediction": np.ascontiguousarray(
                np.asarray(prediction[i], dtype=np.float32).reshape(C, HW)),
            "target": np.ascontiguousarray(
                np.asarray(target[i], dtype=np.float32).reshape(C, HW)),
        })
    res = run_bass_kernel_spmd(_get_nc(), in_maps, core_ids=list(range(B)),
                               trace=trace)
    scores = [_score_from_packed(res.results[i]["out"]) for i in range(B)]
    return np.float32(np.mean(scores)), res


def kernel(prediction, target):
    score, _ = run(prediction, target, trace=False)
    return score


# revision 4
# speedup vs baseline: 1.0247x; 1.0152x over previous
"""ArgmaxIOU kernel v19 for 8 Trainium2 NeuronCores (shipped).

Best measured 133535 ns, rel err 9.5e-04 (fast device period; slow
periods where the DMA fleet drops to ~335 GB/s add ~15us).

v18 = v12 + tail/epilogue trims:
 - eq skips class 20 (row 20 of the eq buffers is constant 1.0; the host
   reconstructs row/col 20 from the pixel row/col sums)
 - tile-3 eq as one 384-col op spanning eq slots e2:e3
 - 3-op final fold on the last tile

Data-parallel over batch: core i processes sample i (B=8, C=21, H=W=512).
Raw Bass (explicit engines + semaphores).

v8 = DVE op-count diet (v7 trace showed DMA at 387 GB/s and DVE as the
new bottleneck at 127.7us busy, bloated by ~110 overhead-dominated
narrow-tile ops):
 - tree21 (6 wide tensor_tensor ops, one sem wait) for every tile
   except the last; quarter-tree only on the final 128-col tile where
   the post-last-byte latency needs it.
 - 512-col eq ops that fill both eq ring slots (subs 2k, 2k+1) in one
   instruction.
 - tiles (512, 512, 512, 384, 128); tile 0 emitted as a small 2-class
   load first so the SDMA doorbell rings early.

Host: gather 8 packed [84,84] matrices, fold G, compute mean IoU.
"""

import sys

import numpy as np

for p in ("/opt/trn_rl_repo",):
    if p not in sys.path:
        sys.path.insert(0, p)

from contextlib import ExitStack

from concourse import bass, mybir
from concourse.bass_utils import run_bass_kernel_spmd

B = 8
C = 21
HW = 512 * 512
P = 128
Q = HW // P              # 2048 pixel columns per partition
G = 4
M = G * C                # 84
EQS = 256                # eq sub-slot columns
TBS = EQS // G           # 64 matmul blocks per eq sub-slot
NEQ = 4                  # eq slot ring depth (even: 512-col eq pair-writes)
NSLOT = 2                # data tile ring depth
SLOTW = 512              # slot width (max tile width)
SLOTE = C * SLOTW

W = [512, 512, 512, 384, 128]
OFF = [0, 512, 1024, 1536, 1920]
NT = len(W)
LAST = NT - 1
QTR = [(0, 5), (5, 10), (10, 15), (15, 21)]

F32 = mybir.dt.float32
BF16 = mybir.dt.bfloat16


def build():
    nc = bass.Bass()
    pred = nc.declare_dram_parameter("prediction", [C, HW], F32, isOutput=False)
    targ = nc.declare_dram_parameter("target", [C, HW], F32, isOutput=False)
    out = nc.declare_dram_parameter("out", [M, M], F32, isOutput=True)

    predv = pred[:].rearrange("c (p q) -> p c q", p=P)
    targv = targ[:].rearrange("c (p q) -> p c q", p=P)

    mx = mybir.AluOpType.max
    eqop = mybir.AluOpType.is_equal

    # ---- load bookkeeping: one sem per load ----
    # tile 0 target: 2 loads (classes 0-2 doorbell + 2-21); last tile:
    # class quarters; everything else: one full-tile load.
    def groups_of(k, tid):
        if k == 0:
            return [(0, 2), (2, 21)] if tid == 0 else [(0, 21)]
        if k == LAST:
            return [(0, 15), (15, 21)]
        return [(0, 10), (10, 21)]

    def loads_of(k, tid):
        return len(groups_of(k, tid))

    # emission order (must match the gpsimd program exactly); the shared
    # dm sem hits 16*(i+1) when load ordinal i (and all before it) is done
    emit_order = []
    for k in (0, 1):
        for tid in (0, 1):
            emit_order += [(k, tid, qi) for qi in range(loads_of(k, tid))]
    for k in (2, 3, 4):
        for tid in (0, 1):
            emit_order += [(k, tid, qi) for qi in range(loads_of(k, tid))]
    sidx = {key: i for i, key in enumerate(emit_order)}

    # ---- eq piece bookkeeping (global columns) ----
    pieces_of = {
        0: [(0, 512)],
        1: [(512, 512)],
        2: [(1024, 512)],
        3: [(1536, 384)],
        4: [(1920, 96), (2016, 32)],
    }

    eqpos = {}
    pos = 0
    for k in range(NT):
        for tid in (0, 1):
            for i in range(len(pieces_of[k])):
                pos += 1
                eqpos[(k, tid, i)] = pos
    n_eq = pos

    eq_done = {k: eqpos[(k, 1, len(pieces_of[k]) - 1)] for k in range(NT)}
    ring_gate = {k: eq_done[k - NSLOT] for k in range(NSLOT, NT)}

    # PE schedule: per sub j, list of (b0, nb, dve_gate)
    NSUB = Q // EQS                      # 8
    pe_sched = {j: [] for j in range(NSUB)}
    for k in range(NT):
        for i, (gcol, glen) in enumerate(pieces_of[k]):
            gate = eqpos[(k, 1, i)]
            j0 = gcol // EQS
            j1 = (gcol + glen - 1) // EQS
            for j in range(j0, j1 + 1):
                lo = max(gcol, j * EQS)
                hi = min(gcol + glen, (j + 1) * EQS)
                pe_sched[j].append(((lo % EQS) // G, (hi - lo) // G, gate))

    with ExitStack() as ctx:
        e_ = ctx.enter_context
        bt = e_(nc.sbuf_tensor("bt", [P, NSLOT * SLOTE], BF16))
        bp = e_(nc.sbuf_tensor("bp", [P, NSLOT * SLOTE], BF16))
        eqt = e_(nc.sbuf_tensor("eqt", [P, NEQ, TBS, C, G], BF16))
        eqp = e_(nc.sbuf_tensor("eqp", [P, NEQ, TBS, C, G], BF16))
        st = e_(nc.sbuf_tensor("st", [P, 10, SLOTW], BF16))
        mo = e_(nc.sbuf_tensor("mo", [P, SLOTW], BF16))
        osb = e_(nc.sbuf_tensor("osb", [M, M], F32))
        conf = e_(nc.psum_tensor("conf", [M, M], F32))
        dms = [e_(nc.semaphore(f"dm{i}")) for i in range(len(emit_order))]
        dve = e_(nc.semaphore("dve"))
        mm = e_(nc.semaphore("mm"))
        fin = e_(nc.semaphore("fin"))
        block = e_(nc.Block())

        srcs = {0: targv, 1: predv}
        pools = {0: bt, 1: bp}

        def tile_view(tid, k):
            s = k % NSLOT
            return pools[tid][:, s * SLOTE:(s + 1) * SLOTE] \
                .rearrange("p (c t) -> p c t", c=C)

        @block.gpsimd
        def _(g):
            def emit(k, tid):
                tv = tile_view(tid, k)
                w = W[k]
                for qi, (clo, chi) in enumerate(groups_of(k, tid)):
                    g.dma_start(
                        out=tv[:, clo:chi, 0:w],
                        in_=srcs[tid][:, clo:chi, OFF[k]:OFF[k] + w],
                    ).then_inc(dms[sidx[(k, tid, qi)]], 16)

            for k in range(NSLOT):
                for tid in (0, 1):
                    emit(k, tid)
            for k in range(NSLOT, NT):
                g.wait_ge(dve, ring_gate[k])
                emit(k, 0)
                emit(k, 1)

        @block.vector
        def _(v):
            def tree21(k, tid):
                # full tile landed: 6 wide ops -> mo[0:w]
                data = tile_view(tid, k)
                w = W[k]
                for qi in range(loads_of(k, tid)):
                    v.wait_ge(dms[sidx[(k, tid, qi)]], 16)
                v.tensor_tensor(st[:, 0:10, 0:w], data[:, 0:10, 0:w],
                                data[:, 10:20, 0:w], mx)
                v.tensor_tensor(st[:, 0:5, 0:w], st[:, 0:5, 0:w], st[:, 5:10, 0:w], mx)
                v.tensor_tensor(st[:, 0:2, 0:w], st[:, 0:2, 0:w], st[:, 2:4, 0:w], mx)
                v.tensor_tensor(st[:, 0:1, 0:w], st[:, 0:1, 0:w], st[:, 1:2, 0:w], mx)
                v.tensor_tensor(st[:, 0:1, 0:w], st[:, 0:1, 0:w], st[:, 4:5, 0:w], mx)
                v.tensor_tensor(mo[:, 0:w].unsqueeze(1), st[:, 0:1, 0:w],
                                data[:, 20:21, 0:w], mx)

            def tree_halves(k, tid):
                # classes 0-9 once the first half-load lands
                data = tile_view(tid, k)
                w = W[k]
                v.wait_ge(dms[sidx[(k, tid, 0)]], 16)
                v.tensor_tensor(st[:, 0:5, 0:w], data[:, 0:5, 0:w], data[:, 5:10, 0:w], mx)
                v.tensor_tensor(st[:, 0:2, 0:w], st[:, 0:2, 0:w], st[:, 2:4, 0:w], mx)
                v.tensor_tensor(st[:, 0:1, 0:w], st[:, 0:1, 0:w], st[:, 1:2, 0:w], mx)
                v.tensor_tensor(st[:, 0:1, 0:w], st[:, 0:1, 0:w], st[:, 4:5, 0:w], mx)
                v.wait_ge(dms[sidx[(k, tid, 1)]], 16)
                v.tensor_tensor(st[:, 1:6, 0:w], data[:, 10:15, 0:w], data[:, 15:20, 0:w], mx)
                v.tensor_tensor(st[:, 1:3, 0:w], st[:, 1:3, 0:w], st[:, 3:5, 0:w], mx)
                v.tensor_tensor(st[:, 1:2, 0:w], st[:, 1:2, 0:w], st[:, 2:3, 0:w], mx)
                v.tensor_tensor(st[:, 1:2, 0:w], st[:, 1:2, 0:w], st[:, 5:6, 0:w], mx)
                v.tensor_tensor(st[:, 1:2, 0:w], st[:, 1:2, 0:w], data[:, 20:21, 0:w], mx)
                v.tensor_tensor(mo[:, 0:w].unsqueeze(1), st[:, 0:1, 0:w],
                                st[:, 1:2, 0:w], mx)

            def tree_tail(k, tid):
                # last tile: classes 0-14 folded while the final 6-class
                # load is in flight; only 4 small ops after the last byte
                data = tile_view(tid, k)
                w = W[k]
                v.wait_ge(dms[sidx[(k, tid, 0)]], 16)
                v.tensor_tensor(st[:, 0:7, 0:w], data[:, 0:7, 0:w], data[:, 7:14, 0:w], mx)
                v.tensor_tensor(st[:, 0:3, 0:w], st[:, 0:3, 0:w], st[:, 3:6, 0:w], mx)
                v.tensor_tensor(st[:, 0:1, 0:w], st[:, 0:1, 0:w], st[:, 1:2, 0:w], mx)
                v.tensor_tensor(st[:, 0:1, 0:w], st[:, 0:1, 0:w], st[:, 2:3, 0:w], mx)
                v.tensor_tensor(st[:, 0:1, 0:w], st[:, 0:1, 0:w], st[:, 6:7, 0:w], mx)
                v.tensor_tensor(st[:, 0:1, 0:w], st[:, 0:1, 0:w], data[:, 14:15, 0:w], mx)
                v.wait_ge(dms[sidx[(k, tid, 1)]], 16)
                v.tensor_tensor(st[:, 1:4, 0:w], data[:, 15:18, 0:w], data[:, 18:21, 0:w], mx)
                v.tensor_tensor(st[:, 0:2, 0:w], st[:, 0:2, 0:w], st[:, 2:4, 0:w], mx)
                v.tensor_tensor(mo[:, 0:w].unsqueeze(1), st[:, 0:1, 0:w],
                                st[:, 1:2, 0:w], mx)

            seen_sub = set()

            def eq_piece(k, tid, i):
                gcol, glen = pieces_of[k][i]
                j0 = gcol // EQS
                j1 = (gcol + glen - 1) // EQS
                need = max((j - (NEQ - 1) for j in range(j0, j1 + 1)
                            if j >= NEQ and j not in seen_sub), default=None)
                for j in range(j0, j1 + 1):
                    seen_sub.add(j)
                if need is not None:
                    v.wait_ge(mm, need)
                data = tile_view(tid, k)
                eqb = eqt if tid == 0 else eqp
                lo = gcol - OFF[k]
                flat = eqb.rearrange("p e tb c g -> p (e tb) c g")
                ep = j0 % NEQ
                b0 = ep * TBS + (gcol % EQS) // G
                nb = glen // G
                outv = flat[:, b0:b0 + nb, 0:20].rearrange("p tb c g -> p c tb g")
                v.tensor_tensor(
                    outv,
                    data[:, 0:20, lo:lo + glen]
                        .rearrange("p c (tb g) -> p c tb g", g=G),
                    mo[:, lo:lo + glen]
                        .rearrange("p (tb g) -> p tb g", g=G)
                        .unsqueeze(1).broadcast_to((P, 20, glen // G, G)),
                    eqop).then_inc(dve, 1)

            ones_done = [False]
            for k in range(NT):
                for tid in (0, 1):
                    if k == LAST:
                        tree_tail(k, tid)
                    elif k == 0:
                        tree21(k, tid)
                    else:
                        tree_halves(k, tid)
                    if not ones_done[0]:
                        ones_done[0] = True
                        onesrc = mo[:, 0:1].unsqueeze(1).unsqueeze(1) \
                            .broadcast_to((P, NEQ, TBS, G))
                        for eqb_ in (eqt, eqp):
                            v.tensor_tensor(eqb_[:, :, :, 20, :], onesrc,
                                            onesrc, mybir.AluOpType.is_ge)
                    for i in range(len(pieces_of[k])):
                        eq_piece(k, tid, i)

            v.wait_ge(mm, NSUB)
            v.tensor_scalar_add(osb[:], conf[:], 0.0).then_inc(dve, 1)

        @block.tensor
        def _(te):
            first = True
            for j in range(NSUB):
                e = j % NEQ
                for (b0, nb, gate) in pe_sched[j]:
                    te.wait_ge(dve, gate)
                    for tb in range(b0, b0 + nb):
                        inst = te.matmul(
                            conf[:],
                            eqt[:, e, tb].rearrange("p c g -> p (c g)"),
                            eqp[:, e, tb].rearrange("p c g -> p (c g)"),
                            start=first,
                            stop=(j == NSUB - 1
                                  and (b0, nb, gate) == pe_sched[j][-1]
                                  and tb == b0 + nb - 1))
                        first = False
                inst.then_inc(mm, 1)

        @block.sync
        def _(sy):
            sy.wait_ge(dve, n_eq + 1)
            sy.dma_start(out=out[:], in_=osb[:]).then_inc(fin, 16)
            sy.wait_ge(fin, 16)

    return nc


def _score_from_packed(packed):
    """packed: [84, 84] f32 -> per-sample mean IoU (float64)."""
    x = packed.astype(np.float64).reshape(C, G, C, G)
    conf = np.einsum("igjg->ij", x)
    A = conf[0:20, 0:20]
    r = conf[0:20, 20] - A.sum(axis=1)
    c = conf[20, 0:20] - A.sum(axis=0)
    t = conf[20, 20] - conf[0:20, 20].sum() - conf[20, 0:20].sum() + A.sum()
    conf[0:20, 20] = r
    conf[20, 0:20] = c
    conf[20, 20] = t
    TP = np.diag(conf).copy()
    FN = conf.sum(axis=1) - TP
    FP = conf.sum(axis=0) - TP
    valid = TP > 0
    denom = TP + FN + FP
    iou = np.where(valid, TP / np.where(valid, denom, 1.0), 0.0)
    n_valid = max(float(valid.sum()), 1.0)
    return iou.sum() / n_valid


_NC_CACHE = {}


def _get_nc():
    if "nc" not in _NC_CACHE:
        _NC_CACHE["nc"] = build()
    return _NC_CACHE["nc"]


def run(prediction, target, trace=False):
    in_maps = []
    for i in range(B):
        in_maps.append({
            "prediction": np.ascontiguousarray(
                np.asarray(prediction[i], dtype=np.float32).reshape(C, HW)),
            "target": np.ascontiguousarray(
                np.asarray(target[i], dtype=np.float32).reshape(C, HW)),
        })
    res = run_bass_kernel_spmd(_get_nc(), in_maps, core_ids=list(range(B)),
                               trace=trace)
    scores = [_score_from_packed(res.results[i]["out"]) for i in range(B)]
    return np.float32(np.mean(scores)), res


def kernel(prediction, target):
    score, _ = run(prediction, target, trace=False)
    return score
